# revision 8
# baseline (speedup 1.0000x reference)
"""Trainium2 Bass kernel for RAFT-style local correlation (sparse_attention).

Math: out[n, g*9+s, h, w] = mean_c f1[n,g*64+c,h,w] * bilinear(f2[n,g*64+c], y, x)
  where x = w + flow_x + (s-4) + eo_x[s],  y = h + flow_y + eo_y[s], zero padding.

Key identity: bilinear sampling commutes with the channel contraction, so
  out = sum_{dy,j} tent(y-(h+dy)) * tent(x-j) * cv[dy,j]
  cv[dy,j] = sum_c f1[c,h,w] * f2[c,h+dy,j]   (integer correlation volume)
Stage 1 computes cv bands via TensorE matmuls (bf16), stage 2 contracts with
separable tent weights via tensor_tensor_reduce on VectorE.

Sharding: 8 cores = 4 batches x 2 H-halves (halo rows of f2 shipped per core).
"""

import numpy as np
import ml_dtypes

import concourse.bass as bass
import concourse.tile as tile
from concourse import bacc
from concourse import mybir
from concourse.bass_utils import run_bass_kernel_spmd

BF16 = mybir.dt.bfloat16
F32 = mybir.dt.float32

N, C, H, W = 4, 256, 64, 256
NG, CG, S = 4, 64, 9
HH = H // 2          # rows per core
NCORE = 8
BLK = 64             # pixel block (psum partition half granularity)


def window_params(DY_LO, DY_HI, D_LO, D_HI):
    JW = BLK + (D_HI - D_LO)              # moving window width per 64-block
    PADX = -D_LO + 1                      # left pad so j index 0 maps >= 0
    WP = W + PADX + D_HI + 1              # padded row width
    NDY = DY_HI - DY_LO + 1
    DYC = max(1, 512 // JW)               # dy per psum chunk (bank limit)
    NCH = -(-NDY // DYC)
    NDYP = NCH * DYC                      # padded dy count (tents are 0 on pad)
    ROWS = HH + NDYP - 1                  # f2 rows per core (zero-padded)
    return dict(JW=JW, PADX=PADX, WP=WP, NDYP=NDYP, DYC=DYC, NCH=NCH, ROWS=ROWS)


def _mk_ap(t_ap, dims, extra_offset=0):
    """Build an AP from a partition-sliced tile AP with custom free dims
    [(stride_elems, count), ...] and an element offset into the free space."""
    ap_list = [list(t_ap.ap[0])] + [[int(s), int(c)] for (s, c) in dims]
    return bass.AP(t_ap.tensor, t_ap.offset + extra_offset, ap_list)


def build_kernel(DY_LO, DY_HI, D_LO, D_HI):
    """Build the SPMD bass graph. Window params are data-derived (host)."""
    p = window_params(DY_LO, DY_HI, D_LO, D_HI)
    JW, WP, NDYP, DYC, NCH, ROWS = (
        p["JW"], p["WP"], p["NDYP"], p["DYC"], p["NCH"], p["ROWS"])

    nc = bacc.Bacc()
    f1p = [nc.declare_dram_parameter(f"f1{i}", [128, HH * W], BF16, isOutput=False)
           for i in range(2)]
    f2p = [nc.declare_dram_parameter(f"f2{i}", [128, ROWS * WP], BF16, isOutput=False)
           for i in range(2)]
    typ = nc.declare_dram_parameter("ty", [HH * 2, 128, S * NDYP], F32, isOutput=False)
    txp = nc.declare_dram_parameter("tx", [HH * 2, 128, S * JW], F32, isOutput=False)
    outp = nc.declare_dram_parameter("out", [HH * 2, 128, NG * S], F32, isOutput=True)

    with tile.TileContext(nc) as tc:
        with (
            tc.tile_pool(name="res", bufs=1) as res,
            tc.tile_pool(name="tw", bufs=3) as tw,
            tc.tile_pool(name="t2", bufs=11) as t2p,
            tc.tile_pool(name="scr", bufs=4) as scr,
            tc.tile_pool(name="ps", bufs=2, space="PSUM") as psp,
        ):
            f1t = [res.tile([128, HH * W], BF16, name=f"f1t{i}", tag=f"f1t{i}") for i in range(2)]
            f2t = [res.tile([128, ROWS * WP], BF16, name=f"f2t{i}", tag=f"f2t{i}") for i in range(2)]
            for i in range(2):
                nc.sync.dma_start(out=f1t[i][:], in_=f1p[i][:, :])
                nc.sync.dma_start(out=f2t[i][:], in_=f2p[i][:, :])
            outacc = res.tile([128, HH * 2 * NG * S], F32, tag="outacc")

            for h in range(HH):
                for sp in range(2):
                    hsp = h * 2 + sp
                    tyt = tw.tile([128, S * NDYP], F32, tag="ty")
                    nc.sync.dma_start(out=tyt[:], in_=typ[hsp, :, :])
                    txt = tw.tile([128, S * JW], F32, tag="tx")
                    nc.sync.dma_start(out=txt[:], in_=txp[hsp, :, :])

                    # T2[s] = ty (x) tx  outer product via broadcast APs
                    t2s = []
                    for s in range(S):
                        t2 = t2p.tile([128, NDYP * JW], F32, tag="t2")
                        ty_ap = _mk_ap(tyt[:], [(1, NDYP), (0, JW)], s * NDYP)
                        tx_ap = _mk_ap(txt[:], [(0, NDYP), (1, JW)], s * JW)
                        o_ap = _mk_ap(t2[:], [(JW, NDYP), (1, JW)])
                        nc.vector.tensor_mul(o_ap, ty_ap, tx_ap)
                        t2s.append(t2)

                    for g in range(NG):
                        half = g // 2          # which 128-channel tensor
                        gp = g % 2             # which 64-partition slice
                        ps = psp.tile([128, NCH * 512], F32, tag="cv")
                        for bb in range(2):    # two 64-px blocks of this sp
                            b = 2 * sp + bb
                            stat = _mk_ap(
                                f1t[half][gp * 64:(gp + 1) * 64, :],
                                [(1, BLK)], h * W + b * BLK)
                            for ci in range(NCH):
                                mov = _mk_ap(
                                    f2t[half][gp * 64:(gp + 1) * 64, :],
                                    [(WP, DYC), (1, JW)],
                                    (h + ci * DYC) * WP + b * BLK + 1)
                                o = _mk_ap(ps[bb * 64:(bb + 1) * 64, :],
                                           [(1, DYC * JW)], ci * 512)
                                nc.tensor.matmul(o, lhsT=stat, rhs=mov,
                                                 start=True, stop=True)
                        # stage 2: per-s tent contraction on psum.
                        # scalar_tensor_tensor (native) per psum chunk:
                        # out=(cv*1/64)*T2, accum=sum; then reduce chunk sums.
                        for s in range(S):
                            ac = scr.tile([128, NCH], F32, tag="ac")
                            for ci in range(NCH):
                                sc = scr.tile([128, DYC * JW], F32, tag="sc")
                                in0 = _mk_ap(ps[:], [(JW, DYC), (1, JW)], ci * 512)
                                t2ap = _mk_ap(t2s[s][:], [(JW, DYC), (1, JW)],
                                              ci * DYC * JW)
                                scap = _mk_ap(sc[:], [(JW, DYC), (1, JW)])
                                nc.vector.scalar_tensor_tensor(
                                    scap, in0, 1.0 / CG, t2ap,
                                    mybir.AluOpType.mult, mybir.AluOpType.mult,
                                    accum_out=ac[:, ci:ci + 1])
                            acc = outacc[:, hsp * NG * S + g * S + s:
                                         hsp * NG * S + g * S + s + 1]
                            nc.vector.tensor_reduce(
                                acc, ac[:], axis=mybir.AxisListType.X,
                                op=mybir.AluOpType.add)

            src = _mk_ap(outacc[:], [(NG * S, HH * 2), (1, NG * S)])
            dst = outp[:, :, :].transpose([1, 0, 2])
            nc.sync.dma_start(out=dst, in_=src)
    return nc


def _prep_core(fmap1, fmap2, v, d, n, half, DY_LO, DY_HI, D_LO, D_HI):
    """Host-side shard prep for one core. v,d are [N,S,H,W] float arrays."""
    p = window_params(DY_LO, DY_HI, D_LO, D_HI)
    JW, PADX, WP, NDYP, ROWS = p["JW"], p["PADX"], p["WP"], p["NDYP"], p["ROWS"]
    h0 = half * HH

    inp = {}
    for i in range(2):
        sl = fmap1[n, i * 128:(i + 1) * 128, h0:h0 + HH, :]
        inp[f"f1{i}"] = np.ascontiguousarray(
            sl.reshape(128, HH * W)).astype(ml_dtypes.bfloat16)
        f2pad = np.zeros((128, ROWS, WP), dtype=ml_dtypes.bfloat16)
        rlo = h0 + DY_LO
        r0 = max(0, -rlo)
        r1 = min(ROWS, H - rlo)
        if r1 > r0:
            f2pad[:, r0:r1, PADX:PADX + W] = fmap2[
                n, i * 128:(i + 1) * 128, rlo + r0:rlo + r1, :]
        inp[f"f2{i}"] = f2pad.reshape(128, ROWS * WP)

    # tent tables: [HH*2, 128, S*NDYP] and [HH*2, 128, S*JW]
    # partition p of set-pair sp -> px = (2*sp + p//64)*64 + p%64
    ty = np.zeros((HH, 2, 128, S, NDYP), dtype=np.float32)
    tx = np.zeros((HH, 2, 128, S, JW), dtype=np.float32)
    vv = v[n, :, h0:h0 + HH, :]          # [S, HH, W]
    dd = d[n, :, h0:h0 + HH, :]
    dy = np.arange(DY_LO, DY_LO + NDYP, dtype=np.float32)
    kk = np.arange(JW, dtype=np.float32)
    for sp in range(2):
        for pb in range(2):
            px = (2 * sp + pb) * 64 + np.arange(64)
            vblk = vv[:, :, px]          # [S, HH, 64]
            dblk = dd[:, :, px]
            t = np.maximum(0.0, 1.0 - np.abs(
                vblk[..., None] - dy[None, None, None, :]))    # [S,HH,64,NDYP]
            ty[:, sp, pb * 64:(pb + 1) * 64, :, :] = t.transpose(1, 2, 0, 3)
            xrel = np.arange(64)[None, None, :] + dblk - D_LO  # rel to window
            t = np.maximum(0.0, 1.0 - np.abs(
                xrel[..., None] - kk[None, None, None, :]))    # [S,HH,64,JW]
            tx[:, sp, pb * 64:(pb + 1) * 64, :, :] = t.transpose(1, 2, 0, 3)
    inp["ty"] = ty.reshape(HH * 2, 128, S * NDYP)
    inp["tx"] = tx.reshape(HH * 2, 128, S * JW)
    return inp


def _host_prep(fmap1, fmap2, flow, extra_offset):
    fmap1 = np.asarray(fmap1, dtype=np.float32)
    fmap2 = np.asarray(fmap2, dtype=np.float32)
    flow = np.asarray(flow, dtype=np.float32)
    eo = np.asarray(extra_offset, dtype=np.float32).reshape(N, S, 2, H, W)

    v = flow[:, None, 1] + eo[:, :, 1]          # [N,S,H,W] y offsets
    u = flow[:, None, 0] + eo[:, :, 0]
    d = u + (np.arange(S, dtype=np.float32) - (S // 2))[None, :, None, None]

    DY_LO = int(np.floor(v.min()))
    DY_HI = int(np.floor(v.max())) + 1
    D_LO = int(np.floor(d.min()))
    D_HI = int(np.floor(d.max())) + 1

    in_maps = []
    for core in range(NCORE):
        n, half = core // 2, core % 2
        in_maps.append(_prep_core(fmap1, fmap2, v, d, n, half,
                                  DY_LO, DY_HI, D_LO, D_HI))
    return (DY_LO, DY_HI, D_LO, D_HI), in_maps


def _unshard(results):
    out = np.zeros((N, NG * S, H, W), dtype=np.float32)
    for core in range(NCORE):
        n, half = core // 2, core % 2
        r = np.asarray(results[core]["out"], dtype=np.float32).reshape(
            HH, 2, 128, NG * S)
        for sp in range(2):
            for pb in range(2):
                px0 = (2 * sp + pb) * 64
                out[n, :, half * HH:(half + 1) * HH, px0:px0 + 64] = \
                    r[:, sp, pb * 64:(pb + 1) * 64, :].transpose(2, 0, 1)
    return out


def kernel(fmap1, fmap2, flow, extra_offset):
    wins, in_maps = _host_prep(fmap1, fmap2, flow, extra_offset)
    nc = build_kernel(*wins)
    if not nc.is_finalized():
        nc.finalize()
    res = run_bass_kernel_spmd(nc, in_maps, core_ids=list(range(NCORE)))
    return _unshard(res.results)
